# revision 1
# baseline (speedup 1.0000x reference)
"""Graphormer multi-head attention on 8 trn2 NeuronCores.

Sharding: sequence-parallel over the 8 sorted batch segments (one graph
per core). Each core runs dense block attention for all 8 heads over its
~512-node segment, padded to a common NB so the program is SPMD.

Formulation (all transposed so softmax reduction rides the matmul
contraction dim):
  S^T[c, r] = K[c, :] . Q[r, :] / sqrt(HD)   (PE, fp32)
  S^T += B^T (edge bias, injected into PSUM via identity matmul)
  P = exp(S^T + colmask)                     (ACT, mask via per-partition bias)
  OT'[d, r] = sum_c V'[c, d] P[c, r]         (PE; V' has a ones column -> row 32
                                              of OT' is the softmax denominator)
  outT = OT'[0:32] * bcast(1/den)            (DVE; bcast via K=1 PE outer product)
  y^T = Wo'^T @ [outT; 1]                    (PE; bias via augmented ones row)
"""

import sys

for _p in ("/opt/trn_rl_repo",):
    if _p not in sys.path:
        sys.path.insert(0, _p)

import numpy as np

import concourse.bass as bass
import concourse.mybir as mybir
import concourse.tile as tile
from concourse.bass_utils import run_bass_kernel_spmd

N, D, H, HD, NCORES = 4096, 256, 8, 32, 8

# ---------------------------------------------------------------------------
# This toolchain's CoreV3 codegen accepts at most ONE semaphore wait per
# engine instruction ("Too many sync wait commands").  Tile freely emits
# several.  Engine queues execute in order, so it is equivalent to hoist all
# but one wait onto single-wait NoOps inserted immediately before the
# instruction on the same engine.  Do that as a BIR-JSON rewrite just before
# neuronxcc compilation.
import json as _json

import concourse.bass2jax as _b2j

_SKIP_OPS = {"EventSemaphore", "UnconditionalBranch", "ConditionalBranch"}


def _split_multiwaits(bir_json: bytes) -> bytes:
    d = _json.loads(bir_json)
    nid = [0]
    for fn in d.get("functions", []):
        for blk in fn.get("blocks", []):
            out = []
            for inst in blk.get("instructions", []):
                si = inst.get("sync_info")
                ow = (si or {}).get("on_wait") or []
                if len(ow) > 1 and inst.get("opcode") not in _SKIP_OPS:
                    for w in ow[:-1]:
                        nid[0] += 1
                        out.append(
                            {
                                "debug": inst.get("debug", 0),
                                "engine": inst["engine"],
                                "ins": [],
                                "name": f"I-waitsplit-{nid[0]}",
                                "opcode": "NoOp",
                                "outs": [],
                                "sync_info": {"on_update": [], "on_wait": [w]},
                            }
                        )
                    si["on_wait"] = [ow[-1]]
                out.append(inst)
            blk["instructions"] = out
    return _json.dumps(d).encode()


_orig_cbk = _b2j.compile_bir_kernel


def _cbk(bir_json, tmpdir, neff_name="file.neff"):
    return _orig_cbk(_split_multiwaits(bir_json), tmpdir, neff_name=neff_name)


if getattr(_b2j.compile_bir_kernel, "__name__", "") != "_cbk":
    _b2j.compile_bir_kernel = _cbk

SCALE = 1.0 / np.sqrt(HD)
NEG = -1.0e9

_prog_cache = {}
_last_in_maps = None


def _build_program(NB):
    NCH = NB // 128
    splits = [(s, min(512, NB - s)) for s in range(0, NB, 512)]
    f32 = mybir.dt.float32
    bf16 = mybir.dt.bfloat16

    nc = bass.Bass()
    xta_d = nc.declare_dram_parameter("xta", [257, NB], f32, isOutput=False)
    w_d = {
        nm: nc.declare_dram_parameter(nm, [257, 256], f32, isOutput=False)
        for nm in ("wqa", "wka", "wva", "woa")
    }
    mask_d = nc.declare_dram_parameter("mask", [128, NCH], f32, isOutput=False)
    ident_d = nc.declare_dram_parameter("ident", [128, 128], f32, isOutput=False)
    bt_d = nc.declare_dram_parameter("bt", [H, NB, NB], f32, isOutput=False)
    yt_d = nc.declare_dram_parameter("yt", [256, NB], f32, isOutput=True)

    kch = [(0, 128), (128, 128), (256, 1)]  # contraction chunks of the 257-row aug

    with tile.TileContext(nc) as tc:
        with (
            tc.tile_pool(name="persist", bufs=1) as pp,
            tc.tile_pool(name="btp", bufs=4) as btp,
            tc.tile_pool(name="pexp", bufs=4) as pxp,
            tc.tile_pool(name="ps_qkv", bufs=1, space="PSUM") as qkvp,
            tc.tile_pool(name="ps_s", bufs=2, space="PSUM") as sp,
            tc.tile_pool(name="ps_o", bufs=1, space="PSUM") as op,
        ):
            # ---- load persistent operands ----
            xt = []
            for k0, kn in kch:
                t = pp.tile([kn, NB], f32, tag=f"xt{k0}", name=f"xt{k0}")
                nc.gpsimd.dma_start(out=t[:], in_=xta_d[k0 : k0 + kn, :])
                xt.append(t)
            wt = {}
            for nm in ("wqa", "wka", "wva", "woa"):
                wt[nm] = []
                for k0, kn in kch:
                    t = pp.tile([kn, 256], f32, tag=f"{nm}{k0}", name=f"{nm}{k0}")
                    nc.gpsimd.dma_start(out=t[:], in_=w_d[nm][k0 : k0 + kn, :])
                    wt[nm].append(t)
            maskt = pp.tile([128, NCH], f32)
            nc.gpsimd.dma_start(out=maskt[:], in_=mask_d[:])
            identt = pp.tile([128, 128], f32)
            nc.gpsimd.dma_start(out=identt[:], in_=ident_d[:])
            ones_row = pp.tile([1, NB], f32)
            nc.vector.memset(ones_row[:], 1.0)

            # ---- Q^T, K^T: 3 tiles per side, heads (0,1,2),(3,4,5),(6,7) so
            # every per-head slice starts at base partition 0/32/64 (PE rule).
            qk_tiles = {}
            for key in ("q", "k"):
                qk_tiles[key] = [
                    pp.tile([min(96, 128), NB], f32, tag=f"{key}g{g}", name=f"{key}g{g}")
                    for g in range(3)
                ]

            def qk_slice(key, h):
                return qk_tiles[key][h // 3][(h % 3) * 32 : (h % 3) * 32 + 32]

            for nm, key, scl in (("wqa", "q", SCALE), ("wka", "k", 1.0)):
                for mg in range(2):
                    acc = qkvp.tile([128, NB], f32, tag="qkv")
                    for fs0, fsn in splits:
                        for ki, (k0, kn) in enumerate(kch):
                            nc.tensor.matmul(
                                acc[:, fs0 : fs0 + fsn],
                                wt[nm][ki][:, mg * 128 : (mg + 1) * 128],
                                xt[ki][:, fs0 : fs0 + fsn],
                                start=(ki == 0),
                                stop=(ki == 2),
                            )
                    for hh in range(4):
                        h = mg * 4 + hh
                        nc.scalar.activation(
                            qk_slice(key, h)[:, :],
                            acc[hh * 32 : (hh + 1) * 32, :],
                            mybir.ActivationFunctionType.Copy,
                            scale=scl,
                        )

            # ---- V natural layout, per 128-row chunk, with ones column ----
            # v33[rc] is [128, 8, 33]: per head 32 value dims + a ones column.
            v33 = []
            for rc in range(NCH):
                dst = pp.tile([128, 8, 33], f32, tag=f"v33_{rc}")
                acc = qkvp.tile([128, 8, 32], f32, tag="qkv")
                for ki, (k0, kn) in enumerate(kch):
                    nc.tensor.matmul(
                        acc[:],
                        xt[ki][:, rc * 128 : (rc + 1) * 128],
                        wt["wva"][ki][:],
                        start=(ki == 0),
                        stop=(ki == 2),
                    )
                nc.vector.tensor_copy(dst[:, :, 0:32], acc[:])
                nc.vector.memset(dst[:, :, 32:33], 1.0)
                v33.append(dst)

            # ---- preload all edge-bias tiles, then one barrier ----
            bt_tiles = {}
            for h in range(H):
                for cc in range(NCH):
                    t = pp.tile([128, NB], f32, tag=f"bt{h}_{cc}", name=f"bt{h}_{cc}")
                    nc.gpsimd.dma_start(
                        out=t[:], in_=bt_d[h, cc * 128 : (cc + 1) * 128, :]
                    )
                    bt_tiles[(h, cc)] = t
            tc.strict_bb_all_engine_barrier()

            # ---- attention per head ----
            outT = [pp.tile([128, NB], f32, tag=f"outT{mg}", name=f"outT{mg}") for mg in range(2)]
            for h in range(H):
                hi, hr = h // 4, (h % 4) * 32
                tc.strict_bb_all_engine_barrier()
                ot = op.tile([33, NB], f32, tag="ot")
                for cc in range(NCH):
                    bt_t = bt_tiles[(h, cc)]
                    p_t = pxp.tile([128, NB], f32, tag="p")
                    s_t = sp.tile([128, NB], f32, tag="s")
                    for fs0, fsn in splits:
                        nc.tensor.matmul(
                            s_t[:, fs0 : fs0 + fsn],
                            qk_slice("k", h)[:, cc * 128 : (cc + 1) * 128],
                            qk_slice("q", h)[:, fs0 : fs0 + fsn],
                            start=True,
                            stop=False,
                        )
                        nc.tensor.matmul(
                            s_t[:, fs0 : fs0 + fsn],
                            identt[:],
                            bt_t[:, fs0 : fs0 + fsn],
                            start=False,
                            stop=True,
                        )
                    nc.scalar.activation(
                        p_t[:],
                        s_t[:],
                        mybir.ActivationFunctionType.Exp,
                        bias=maskt[:, cc : cc + 1],
                        scale=1.0,
                    )
                    for fs0, fsn in splits:
                        nc.tensor.matmul(
                            ot[:, fs0 : fs0 + fsn],
                            v33[cc][:, h, :],
                            p_t[:, fs0 : fs0 + fsn],
                            start=(cc == 0),
                            stop=(cc == NCH - 1),
                        )
                # normalize: row 32 of ot is the denominator
                recip = pxp.tile([1, NB], f32, tag="recip")
                nc.vector.reciprocal(recip[:], ot[32:33, :])
                rb = sp.tile([32, NB], f32, tag="s", name="rb")
                for fs0, fsn in splits:
                    nc.tensor.matmul(
                        rb[:, fs0 : fs0 + fsn],
                        ones_row[0:1, 0:32],
                        recip[:, fs0 : fs0 + fsn],
                        start=True,
                        stop=True,
                    )
                rb_sb = pxp.tile([32, NB], f32, tag="rb_sb")
                nc.scalar.activation(rb_sb[:], rb[:], mybir.ActivationFunctionType.Copy)
                nc.vector.tensor_mul(
                    outT[hi][hr : hr + 32, :], ot[0:32, :], rb_sb[:]
                )

            # ---- final projection y^T = Wo'^T @ [outT; 1] ----
            out_k = [outT[0], outT[1], ones_row]
            for mg in range(2):
                dst = pp.tile([128, NB], f32, tag=f"yt{mg}", name=f"yts{mg}")
                acc = qkvp.tile([128, NB], f32, tag="qkv")
                for fs0, fsn in splits:
                    for ki in range(3):
                        nc.tensor.matmul(
                            acc[:, fs0 : fs0 + fsn],
                            wt["woa"][ki][:, mg * 128 : (mg + 1) * 128],
                            out_k[ki][:, fs0 : fs0 + fsn] if ki < 2
                            else ones_row[0:1, fs0 : fs0 + fsn],
                            start=(ki == 0),
                            stop=(ki == 2),
                        )
                nc.scalar.activation(
                    dst[:], acc[:], mybir.ActivationFunctionType.Copy
                )
                nc.gpsimd.dma_start(out=yt_d[mg * 128 : (mg + 1) * 128, :], in_=dst[:])

    return nc


def kernel(x, edge_index, edge_attr, batch, Wq, bq, Wk, bk, Wv, bv, Wo, bo, We, be):
    x = np.asarray(x, np.float32)
    edge_index = np.asarray(edge_index)
    edge_attr = np.asarray(edge_attr, np.float32)
    batch = np.asarray(batch)
    n = x.shape[0]

    counts = np.bincount(batch.astype(np.int64), minlength=NCORES)
    starts = np.concatenate([[0], np.cumsum(counts)])[:NCORES]
    NB = max(640, int(-(-counts.max() // 128)) * 128)

    wq_a = np.vstack([np.asarray(Wq, np.float32), np.asarray(bq, np.float32)[None]])
    wk_a = np.vstack([np.asarray(Wk, np.float32), np.asarray(bk, np.float32)[None]])
    wv_a = np.vstack([np.asarray(Wv, np.float32), np.asarray(bv, np.float32)[None]])
    wo_a = np.vstack([np.asarray(Wo, np.float32), np.asarray(bo, np.float32)[None]])
    ident = np.eye(128, dtype=np.float32)

    # edge bias values and per-core dense bias blocks (scatter on host for now)
    eb = edge_attr @ np.asarray(We, np.float32) + np.asarray(be, np.float32)  # [E,H]
    r_all, c_all = edge_index[0], edge_index[1]
    br, bc = batch[r_all], batch[c_all]

    in_maps = []
    for b in range(NCORES):
        s0, nb = int(starts[b]), int(counts[b])
        xta = np.zeros((257, NB), np.float32)
        xta[:256, :nb] = x[s0 : s0 + nb].T
        xta[256, :] = 1.0
        mask = np.zeros((NB,), np.float32)
        mask[nb:] = NEG
        sel = np.where((br == b) & (bc == b))[0]
        rl = (r_all[sel] - s0).astype(np.int64)
        cl = (c_all[sel] - s0).astype(np.int64)
        bt = np.zeros((H, NB, NB), np.float32)
        for h in range(H):
            np.add.at(bt[h], (cl, rl), eb[sel, h])
        in_maps.append(
            {
                "xta": xta,
                "wqa": wq_a,
                "wka": wk_a,
                "wva": wv_a,
                "woa": wo_a,
                "mask": np.ascontiguousarray(mask.reshape(NB // 128, 128).T),
                "ident": ident,
                "bt": bt,
            }
        )

    key = NB
    if key not in _prog_cache:
        _prog_cache[key] = _build_program(NB)
    nc = _prog_cache[key]

    global _last_in_maps
    _last_in_maps = in_maps
    res = run_bass_kernel_spmd(nc, in_maps, list(range(NCORES)))
    y = np.empty((n, D), np.float32)
    for b in range(NCORES):
        s0, nb = int(starts[b]), int(counts[b])
        y[s0 : s0 + nb] = res.results[b]["yt"][:, :nb].T
    return y



# revision 2
# speedup vs baseline: 5159.1640x; 5159.1640x over previous
"""Graphormer multi-head attention on 8 trn2 NeuronCores — v2.

Sharding: sequence-parallel over the 8 sorted batch segments (one graph
per core). Each core runs dense block attention for all 8 heads over its
~512-node segment, padded to a common NB so the program is SPMD.

v2 changes vs the staged baseline:
  - The dense [H, NB, NB] edge-bias tensor (13.1 MB/core, the dominant
    per-call input traffic) is replaced by a ~57 KB COO tensor. On device,
    one-hot row/col matrices are generated with iota + is_equal on DVE and
    the bias is scattered into the score PSUM accumulation with extra
    matmuls (S^T group: K.Q matmul + per-edge-chunk Cv^T.R matmuls).
  - All matmul operands are fp16 instead of fp32: 4x faster on the PE and
    half the input bytes. Accumulation stays fp32 in PSUM.
  - Inputs consolidated into 4 DRAM tensors (~1 MB/core vs 14.8 MB).
  - fp16 output, converted to fp32 on host.

Formulation (transposed so the softmax reduction rides the matmul
contraction dim):
  S^T[c, r] = K[c, :] . Q[r, :] / sqrt(HD) + sum_e C[e, c] v_h[e] R[e, r]
  P = exp(S^T + colmask)                     (ACT, mask via per-partition bias)
  OT'[d, r] = sum_c V'[c, d] P[c, r]         (PE; V' has a ones column -> row 32
                                              of OT' is the softmax denominator)
  outT = OT'[0:32] * bcast(1/den)            (DVE; bcast via K=1 PE outer product)
  y^T = Wo'^T @ [outT; 1]                    (PE; bias via augmented ones row)
"""

import sys

for _p in ("/opt/trn_rl_repo",):
    if _p not in sys.path:
        sys.path.insert(0, _p)

import numpy as np

import concourse.bass as bass
import concourse.mybir as mybir
import concourse.tile as tile
from concourse.bass_utils import run_bass_kernel_spmd

N, D, H, HD, NCORES = 4096, 256, 8, 32, 8

# ---------------------------------------------------------------------------
# This toolchain's CoreV3 codegen accepts at most ONE semaphore wait per
# engine instruction ("Too many sync wait commands").  Tile freely emits
# several.  Engine queues execute in order, so it is equivalent to hoist all
# but one wait onto single-wait NoOps inserted immediately before the
# instruction on the same engine.  Do that as a BIR-JSON rewrite just before
# neuronxcc compilation.
import json as _json

import concourse.bass2jax as _b2j

_SKIP_OPS = {"EventSemaphore", "UnconditionalBranch", "ConditionalBranch"}


def _split_multiwaits(bir_json: bytes) -> bytes:
    d = _json.loads(bir_json)
    nid = [0]
    for fn in d.get("functions", []):
        for blk in fn.get("blocks", []):
            out = []
            for inst in blk.get("instructions", []):
                si = inst.get("sync_info")
                ow = (si or {}).get("on_wait") or []
                if len(ow) > 1 and inst.get("opcode") not in _SKIP_OPS:
                    for w in ow[:-1]:
                        nid[0] += 1
                        out.append(
                            {
                                "debug": inst.get("debug", 0),
                                "engine": inst["engine"],
                                "ins": [],
                                "name": f"I-waitsplit-{nid[0]}",
                                "opcode": "NoOp",
                                "outs": [],
                                "sync_info": {"on_update": [], "on_wait": [w]},
                            }
                        )
                    si["on_wait"] = [ow[-1]]
                out.append(inst)
            blk["instructions"] = out
    return _json.dumps(d).encode()


_orig_cbk = _b2j.compile_bir_kernel


def _cbk(bir_json, tmpdir, neff_name="file.neff"):
    return _orig_cbk(_split_multiwaits(bir_json), tmpdir, neff_name=neff_name)


if getattr(_b2j.compile_bir_kernel, "__name__", "") != "_cbk":
    _b2j.compile_bir_kernel = _cbk

SCALE = 1.0 / np.sqrt(HD)
NEG = -1.0e9

_prog_cache = {}
_last_in_maps = None


def _build_program(NB, EC):
    """EC: tuple, EC[cc] = number of 128-edge slot chunks for column chunk cc."""
    NCH = NB // 128
    assert len(EC) == NCH
    TOTCH = sum(EC)
    base_of = np.concatenate([[0], np.cumsum(EC)]).astype(int)
    splits = [(s, min(512, NB - s)) for s in range(0, NB, 512)]
    f32 = mybir.dt.float32
    f16 = mybir.dt.float16
    XWW = NB + 1024  # x^T-aug columns then wq|wk|wv|wo blocks

    nc = bass.Bass()
    xw_d = nc.declare_dram_parameter("xw", [257, XWW], f16, isOutput=False)
    ed_d = nc.declare_dram_parameter("ed", [128, max(TOTCH, 1), 10], f32, isOutput=False)
    mask_d = nc.declare_dram_parameter("mask", [128, NCH], f32, isOutput=False)
    yt_d = nc.declare_dram_parameter("yt", [256, NB], f16, isOutput=True)

    with tile.TileContext(nc) as tc:
        with (
            tc.tile_pool(name="persist", bufs=1) as pp,
            tc.tile_pool(name="pexp", bufs=4) as pxp,
            tc.tile_pool(name="ps_qkv", bufs=1, space="PSUM") as psQ,
            tc.tile_pool(name="ps_s", bufs=2, space="PSUM") as psS,
            tc.tile_pool(name="ps_o", bufs=1, space="PSUM") as psO,
        ):
            # ---- load inputs ----
            xw = [
                pp.tile([128, XWW], f16, tag="xw0", name="xw0"),
                pp.tile([128, XWW], f16, tag="xw1", name="xw1"),
                pp.tile([1, XWW], f16, tag="xw2", name="xw2"),
            ]
            nc.gpsimd.dma_start(out=xw[0][:], in_=xw_d[0:128, :])
            nc.gpsimd.dma_start(out=xw[1][:], in_=xw_d[128:256, :])
            nc.gpsimd.dma_start(out=xw[2][:], in_=xw_d[256:257, :])
            edt = pp.tile([128, max(TOTCH, 1), 10], f32, tag="edt", name="edt")
            nc.gpsimd.dma_start(out=edt[:], in_=ed_d[:])
            maskt = pp.tile([128, NCH], f32, tag="mask", name="mask")
            nc.gpsimd.dma_start(out=maskt[:], in_=mask_d[:])

            kch = [(0, 128), (1, 128), (2, 1)]  # (xw tile idx, contraction rows)

            def xt(ki):
                return xw[ki][:, 0:NB]

            def wslice(nm_i, ki):
                b = NB + nm_i * 256
                return xw[ki][:, b : b + 256]

            ones_row = xt(2)  # [1, NB] of 1.0 (augmented row of x^T)

            # ---- iota for one-hot generation ----
            iota_t = pp.tile([128, NB], f32, tag="iota", name="iota")
            nc.gpsimd.iota(
                iota_t[:],
                pattern=[[1, NB]],
                base=0,
                channel_multiplier=0,
                allow_small_or_imprecise_dtypes=True,
            )

            # ---- R one-hots [slot, r] (shared across heads) ----
            Rt = []
            for k in range(TOTCH):
                t = pp.tile([128, NB], f16, tag=f"R{k}", name=f"R{k}")
                nc.vector.tensor_scalar(
                    t[:],
                    iota_t[:],
                    edt[:, k, 0:1],
                    None,
                    op0=mybir.AluOpType.is_equal,
                )
                Rt.append(t)

            # ---- Cv one-hots [slot, c] * v_h, head-major so head 0 is ready
            # early ----
            cvt = [[None] * TOTCH for _ in range(H)]
            for h in range(H):
                for k in range(TOTCH):
                    t = pp.tile([128, 128], f16, tag=f"cv{h}_{k}", name=f"cv{h}_{k}")
                    nc.vector.tensor_scalar(
                        t[:],
                        iota_t[:, 0:128],
                        edt[:, k, 1:2],
                        edt[:, k, 2 + h : 3 + h],
                        op0=mybir.AluOpType.is_equal,
                        op1=mybir.AluOpType.mult,
                    )
                    cvt[h][k] = t

            # ---- Q^T, K^T: 3 tiles per side, heads (0,1,2),(3,4,5),(6,7) so
            # every per-head slice starts at base partition 0/32/64 (PE rule).
            qk_tiles = {}
            for key in ("q", "k"):
                qk_tiles[key] = [
                    pp.tile([96, NB], f16, tag=f"{key}g{g}", name=f"{key}g{g}")
                    for g in range(3)
                ]

            def qk_slice(key, h):
                return qk_tiles[key][h // 3][(h % 3) * 32 : (h % 3) * 32 + 32]

            for nm_i, key, scl in ((0, "q", SCALE), (1, "k", 1.0)):
                for mg in range(2):
                    acc = psQ.tile([128, NB], f32, tag="acc")
                    for fs0, fsn in splits:
                        for ci, (ki, kn) in enumerate(kch):
                            nc.tensor.matmul(
                                acc[:, fs0 : fs0 + fsn],
                                wslice(nm_i, ki)[:, mg * 128 : (mg + 1) * 128],
                                xt(ki)[:, fs0 : fs0 + fsn],
                                start=(ci == 0),
                                stop=(ci == 2),
                            )
                    for hh in range(4):
                        h = mg * 4 + hh
                        nc.scalar.activation(
                            qk_slice(key, h)[:, :],
                            acc[hh * 32 : (hh + 1) * 32, :],
                            mybir.ActivationFunctionType.Copy,
                            scale=scl,
                        )

            # ---- V natural layout, per 128-row chunk, with ones column ----
            v33 = []
            for rc in range(NCH):
                dst = pp.tile([128, 8, 33], f16, tag=f"v33_{rc}", name=f"v33_{rc}")
                acc = psQ.tile([128, 8, 32], f32, tag="acc")
                for ci, (ki, kn) in enumerate(kch):
                    nc.tensor.matmul(
                        acc[:],
                        xt(ki)[:, rc * 128 : (rc + 1) * 128],
                        wslice(2, ki)[:],
                        start=(ci == 0),
                        stop=(ci == 2),
                    )
                nc.vector.tensor_copy(dst[:, :, 0:32], acc[:])
                nc.vector.memset(dst[:, :, 32:33], 1.0)
                v33.append(dst)

            # ---- attention per head ----
            outT = [
                pp.tile([128, NB], f16, tag=f"outT{mg}", name=f"outT{mg}")
                for mg in range(2)
            ]
            for h in range(H):
                hi, hr = h // 4, (h % 4) * 32
                ot = psO.tile([33, NB], f32, tag="ot")
                for cc in range(NCH):
                    ec = EC[cc]
                    p_t = pxp.tile([128, NB], f16, tag="p")
                    s_t = psS.tile([128, NB], f32, tag="s")
                    for fs0, fsn in splits:
                        nc.tensor.matmul(
                            s_t[:, fs0 : fs0 + fsn],
                            qk_slice("k", h)[:, cc * 128 : (cc + 1) * 128],
                            qk_slice("q", h)[:, fs0 : fs0 + fsn],
                            start=True,
                            stop=(ec == 0),
                        )
                        for j in range(ec):
                            k_idx = int(base_of[cc]) + j
                            nc.tensor.matmul(
                                s_t[:, fs0 : fs0 + fsn],
                                cvt[h][k_idx][:],
                                Rt[k_idx][:, fs0 : fs0 + fsn],
                                start=False,
                                stop=(j == ec - 1),
                            )
                    nc.scalar.activation(
                        p_t[:],
                        s_t[:],
                        mybir.ActivationFunctionType.Exp,
                        bias=maskt[:, cc : cc + 1],
                        scale=1.0,
                    )
                    for fs0, fsn in splits:
                        nc.tensor.matmul(
                            ot[:, fs0 : fs0 + fsn],
                            v33[cc][:, h, :],
                            p_t[:, fs0 : fs0 + fsn],
                            start=(cc == 0),
                            stop=(cc == NCH - 1),
                        )
                # normalize: row 32 of ot is the denominator
                recip = pxp.tile([1, NB], f16, tag="recip")
                with nc.allow_low_precision(reason="softmax denom recip in f16"):
                    nc.vector.reciprocal(recip[:], ot[32:33, :])
                rb = psS.tile([32, NB], f32, tag="s", name=f"rb{h}")
                for fs0, fsn in splits:
                    nc.tensor.matmul(
                        rb[:, fs0 : fs0 + fsn],
                        ones_row[0:1, 0:32],
                        recip[:, fs0 : fs0 + fsn],
                        start=True,
                        stop=True,
                    )
                rb_sb = pxp.tile([32, NB], f32, tag="rb_sb")
                nc.vector.tensor_copy(rb_sb[:], rb[:])
                nc.vector.tensor_tensor(
                    outT[hi][hr : hr + 32, :],
                    ot[0:32, :],
                    rb_sb[:],
                    op=mybir.AluOpType.mult,
                )

            # ---- final projection y^T = Wo'^T @ [outT; 1] ----
            out_k = [outT[0], outT[1], ones_row]
            for mg in range(2):
                dst = pp.tile([128, NB], f16, tag=f"yt{mg}", name=f"yts{mg}")
                acc = psQ.tile([128, NB], f32, tag="acc")
                for fs0, fsn in splits:
                    for ki in range(3):
                        nc.tensor.matmul(
                            acc[:, fs0 : fs0 + fsn],
                            wslice(3, ki)[:, mg * 128 : (mg + 1) * 128],
                            out_k[ki][:, fs0 : fs0 + fsn]
                            if ki < 2
                            else ones_row[0:1, fs0 : fs0 + fsn],
                            start=(ki == 0),
                            stop=(ki == 2),
                        )
                nc.scalar.activation(
                    dst[:], acc[:], mybir.ActivationFunctionType.Copy
                )
                nc.gpsimd.dma_start(out=yt_d[mg * 128 : (mg + 1) * 128, :], in_=dst[:])

    return nc


def kernel(x, edge_index, edge_attr, batch, Wq, bq, Wk, bk, Wv, bv, Wo, bo, We, be):
    x = np.asarray(x, np.float32)
    edge_index = np.asarray(edge_index).astype(np.int64)
    edge_attr = np.asarray(edge_attr, np.float32)
    batch = np.asarray(batch).astype(np.int64)
    n = x.shape[0]

    counts = np.bincount(batch, minlength=NCORES)
    starts = np.concatenate([[0], np.cumsum(counts)])[:NCORES]
    NB = max(128, int(-(-counts.max() // 128)) * 128)
    NCH = NB // 128

    # edge bias values; only within-graph edges matter (rest are masked)
    eb = edge_attr @ np.asarray(We, np.float32) + np.asarray(be, np.float32)  # [E,H]
    r_all, c_all = edge_index[0], edge_index[1]
    br, bc = batch[r_all], batch[c_all]

    per_core = []
    maxec = [0] * NCH
    for b in range(NCORES):
        s0 = int(starts[b])
        sel = np.where((br == b) & (bc == b))[0]
        rl = (r_all[sel] - s0).astype(np.int64)
        cl = (c_all[sel] - s0).astype(np.int64)
        vals = eb[sel]
        groups = []
        for cc in range(NCH):
            m = (cl // 128) == cc
            g = (rl[m], cl[m] - cc * 128, vals[m])
            groups.append(g)
            maxec[cc] = max(maxec[cc], -(-len(g[0]) // 128))
        per_core.append(groups)
    EC = tuple(maxec)
    TOTCH = sum(EC)
    base_of = np.concatenate([[0], np.cumsum(EC)]).astype(int)

    # ---- build per-core input tensors ----
    XWW = NB + 1024
    wq_a = np.vstack([np.asarray(Wq, np.float32), np.asarray(bq, np.float32)[None]])
    wk_a = np.vstack([np.asarray(Wk, np.float32), np.asarray(bk, np.float32)[None]])
    wv_a = np.vstack([np.asarray(Wv, np.float32), np.asarray(bv, np.float32)[None]])
    wo_a = np.vstack([np.asarray(Wo, np.float32), np.asarray(bo, np.float32)[None]])
    wblk = np.concatenate([wq_a, wk_a, wv_a, wo_a], axis=1)  # [257, 1024]

    in_maps = []
    for b in range(NCORES):
        s0, nb = int(starts[b]), int(counts[b])
        xwt = np.zeros((257, XWW), np.float16)
        xwt[:256, :nb] = x[s0 : s0 + nb].T.astype(np.float16)
        xwt[256, :NB] = 1.0
        xwt[:, NB:] = wblk.astype(np.float16)

        ed = np.zeros((128, max(TOTCH, 1), 10), np.float32)
        ed[:, :, 0] = -1.0
        ed[:, :, 1] = -1.0
        for cc in range(NCH):
            rl, cw, vals = per_core[b][cc]
            for j in range(len(rl)):
                k = int(base_of[cc]) + j // 128
                p = j % 128
                ed[p, k, 0] = rl[j]
                ed[p, k, 1] = cw[j]
                ed[p, k, 2:10] = vals[j]

        mask = np.zeros((NB,), np.float32)
        mask[nb:] = NEG
        in_maps.append(
            {
                "xw": xwt,
                "ed": ed,
                "mask": np.ascontiguousarray(mask.reshape(NCH, 128).T),
            }
        )

    key = (NB, EC)
    if key not in _prog_cache:
        _prog_cache[key] = _build_program(NB, EC)
    nc = _prog_cache[key]

    global _last_in_maps
    _last_in_maps = in_maps
    res = run_bass_kernel_spmd(nc, in_maps, list(range(NCORES)))
    y = np.empty((n, D), np.float32)
    for b in range(NCORES):
        s0, nb = int(starts[b]), int(counts[b])
        y[s0 : s0 + nb] = res.results[b]["yt"][:, :nb].T.astype(np.float32)
    return y
